# revision 24
# baseline (speedup 1.0000x reference)
# DSTP-RNN Trainium2 kernel: 8-core pure data parallel (batch 512 -> 64/core).
#
# Restructuring summary (validated numerically vs fp32 ref):
#  - "Score" tensors are b-major: partitions = (g, b) with g in {0,1} a
#    channel-group split, b = 64 local batch rows; free dims = (ch, tau).
#  - Per-step attention score: DVE broadcast-add of e, ACT tanh, DVE mul by
#    replicated v, DVE pairwise tree-reduce over tau (all bf16).
#  - Softmax without max-subtraction (scores are small); channel-group fold
#    and per-partition normalizer duplication via tiny PE matmuls.
#  - LSTM is H-major: gates in psum [128H, 4, 64b] built from per-gate
#    matmul chains (weights are the moving operand, all bf16 => 1 cyc/row);
#    h/c states stay [128H, 64b] so no per-step transposes are needed.
#    Gate order host-permuted to [i,f,o | g]; sigmoid via tanh(x/2) with the
#    0.5 folded into weights; states doubled (hS=2h, cS=2c).
#  - Stage-3 context vector din = sum_tau a*final via 64 per-b tiny PE
#    matmuls (stationary = final_b, moving = a_b column), out free size 1.
#  - All cross-partition movement via PE (transpose matmuls with identity,
#    fold/dup matmuls with 0/1 matrices); DVE/ACT stay lane-aligned.
import numpy as np
import ml_dtypes

import concourse.bacc as bacc
import concourse.mybir as mybir
import concourse.tile as tile
from concourse.bass_utils import run_bass_kernel_spmd

F32 = mybir.dt.float32
BF16 = mybir.dt.bfloat16
AX = mybir.AxisListType
OP = mybir.AluOpType
AF = mybir.ActivationFunctionType

N_CORES = 8
B = 64      # batch per core
T = 64      # encoder length
H = 128
TD = 24     # decoder steps (T_DEC + 6)
NF = 17     # driving series count
C2 = 129    # stage-2 channels (H + label)
COLS = np.array(list(range(14)) + list(range(15, 18)))
PAD_NEG = -20.0   # pad channel fill (tanh -> -1; excluded from softmax sums)


def _perm_cols(w):
    # torch gate order (i,f,g,o) -> (i,f,o,g): sigmoid block contiguous
    i, f, g, o = np.split(w, 4, axis=-1)
    return np.concatenate([i, f, o, g], axis=-1)


def _bf(x):
    return np.ascontiguousarray(np.asarray(x).astype(ml_dtypes.bfloat16))


def _f32(x):
    return np.ascontiguousarray(np.asarray(x).astype(np.float32))


def prep_weights(inp):
    w = {}
    w["Wi1R"] = _bf(np.concatenate([inp["Wi_w"].T, inp["Wi_b"][None, :]], 0))
    w["Wi2R"] = _bf(np.concatenate([inp["Wi2_w"].T * 0.5, inp["Wi2_b"][None, :]], 0))
    w["We1R"] = _bf(inp["We_w"].T * 0.5)
    w["We2R"] = _bf(inp["We2_w"].T * 0.5)
    w["WhR"] = _bf(inp["Wh_w"].T * 0.5)
    w["WxR"] = _bf(inp["Wx_w"].T * 0.5)
    w["Wxb"] = _bf(inp["Wx_b"][None, :])

    # ISO: sigmoid gates computed as tanh(x/2) -> pre-scale i,f,o cols by 0.5.
    # States are stored doubled (hS=2h, cS=2c), so weight blocks consuming
    # h/c/mid/din get an extra 0.5.
    ISO = np.concatenate([0.5 * np.ones(384), np.ones(128)]).astype(np.float32)
    g1x = _perm_cols(inp["Wih1"].T) * ISO
    b1 = _perm_cols((inp["bih1"] + inp["bhh1"])[None, :]) * ISO
    w["G1XA"] = _bf(np.concatenate([g1x[0:9], b1], 0))
    w["G1XB"] = _bf(g1x[9:17])
    w["G1H"] = _bf(_perm_cols(inp["Whh1"].T) * ISO * 0.5)

    # stage-2 channel order: chA = [h0..h63, label], chB = [h64..h127]
    g2x = _perm_cols(inp["Wih2"].T) * ISO * 0.5
    b2 = _perm_cols((inp["bih2"] + inp["bhh2"])[None, :]) * ISO
    w["G2XA"] = _bf(np.concatenate([g2x[0:64], np.zeros((1, 512), np.float32), b2], 0))
    w["G2XB"] = _bf(np.concatenate([g2x[64:128], g2x[128:129]], 0))
    w["G2H"] = _bf(_perm_cols(inp["Whh2"].T) * ISO * 0.5)

    w["GdX"] = _bf(_perm_cols(inp["Wihd"].T) * ISO * 0.5)
    w["GdH"] = _bf(_perm_cols(inp["Whhd"].T) * ISO * 0.5)
    w["bdrow"] = _bf(_perm_cols((inp["bihd"] + inp["bhhd"])[None, :]) * ISO)

    w["vdup1"] = _bf(np.broadcast_to(inp["Vd_w"][0][None, :], (128, T)))
    w["vdup2"] = _bf(np.broadcast_to(inp["Vd2_w"][0][None, :], (128, T)))
    w["vdup3"] = _bf(np.broadcast_to(inp["V_w"][0][None, :], (128, H)))
    w["regw"] = _bf(inp["reg_w"][0][:, None] * 0.5)

    eye = np.eye(64, dtype=np.float32)
    w["I64dup"] = _f32(np.concatenate([eye, eye], 0))
    w["I128bf"] = _bf(np.eye(128, dtype=np.float32))
    foldDup = (np.arange(128)[:, None] % 64 == np.arange(128)[None, :] % 64)
    w["foldDup"] = _f32(foldDup.astype(np.float32))
    return w


def prep_core_inputs(inp, core):
    b0, b1 = core * B, (core + 1) * B
    x = np.asarray(inp["input_p_q"])[b0:b1, :T, :][:, :, COLS]   # [64,64,17]
    lab = np.asarray(inp["label_p"])[b0:b1, :T]                  # [64,64]
    d = {}
    inpT = np.ones((65, NF * B), np.float32)
    inpT[:64] = x.transpose(1, 2, 0).reshape(64, NF * B)         # [t, (c,b)]
    d["inpT"] = _bf(inpT)
    ct = x.transpose(2, 1, 0).reshape(NF, T * B)                 # [c, (t,b)]
    ct2 = np.zeros((10, 2, T * B), np.float32)
    ct2[0:9, 0] = ct[0:9]
    ct2[0:8, 1] = ct[9:17]
    ct2[9, 0] = 1.0
    d["inpCT2"] = _bf(ct2.reshape(10, 2 * T * B))
    d["labelT"] = _f32(lab.T * 2.0)                                    # [t, b]
    return d


DRAM_SPECS = {
    "inpT": ([65, NF * B], BF16), "inpCT2": ([10, 2 * T * B], BF16),
    "labelT": ([T, B], F32),
    "Wi1R": ([65, 64], BF16), "Wi2R": ([65, 64], BF16),
    "We1R": ([256, 64], BF16), "We2R": ([256, 64], BF16),
    "WhR": ([256, 128], BF16), "WxR": ([128, 128], BF16), "Wxb": ([1, 128], BF16),
    "G1XA": ([10, 512], BF16), "G1XB": ([8, 512], BF16), "G1H": ([128, 512], BF16),
    "G2XA": ([66, 512], BF16), "G2XB": ([65, 512], BF16), "G2H": ([128, 512], BF16),
    "GdX": ([128, 512], BF16), "GdH": ([128, 512], BF16), "bdrow": ([1, 512], BF16),
    "vdup1": ([128, T], BF16), "vdup2": ([128, T], BF16), "vdup3": ([128, H], BF16),
    "regw": ([128, 1], BF16), "I64dup": ([128, 64], F32), "I128bf": ([128, 128], BF16),
    "foldDup": ([128, 128], F32),
}


def build_nc(num_devices=N_CORES, skip_score=False, skip_tail=False, only_stages=(1, 2, 3), split=0.42, split3=None, pool2=0, pool3=0):
    nc = bacc.Bacc("TRN2", target_bir_lowering=False, debug=False,
                   num_devices=num_devices)
    dr = {}
    for name, (shape, dt) in DRAM_SPECS.items():
        dr[name] = nc.dram_tensor(name, shape, dt, kind="ExternalInput").ap()
    out_d = nc.dram_tensor("out", [B, 18], F32, kind="ExternalOutput").ap()

    with tile.TileContext(nc) as tc:
        # ---------- persistent SBUF ----------
        wpool = tc.alloc_tile_pool(name="wpool", bufs=1)
        sb = {}
        for name, (shape, dt) in DRAM_SPECS.items():
            if shape[0] > 128:
                assert shape[0] == 256
                for half, suf in ((0, "a"), (1, "b")):
                    key = name + suf
                    sb[key] = wpool.tile([128, shape[1]], dt, name=f"sb_{key}")
                    nc.sync.dma_start(sb[key][:], dr[name][128 * half:128 * (half + 1), :])
            elif name == "inpCT2":
                sb[name] = wpool.tile([10, 2, T * B], dt, name=f"sb_{name}")
                nc.sync.dma_start(sb[name][:, 0, :], dr[name][:, 0:T * B])
                nc.sync.dma_start(sb[name][:, 1, :], dr[name][:, T * B:])
            else:
                sb[name] = wpool.tile(shape, dt, name=f"sb_{name}")
                nc.sync.dma_start(sb[name][:], dr[name][:])

        X1 = wpool.tile([128, 9, T], BF16, name="X1")
        X2 = wpool.tile([128, 65, T], BF16, name="X2")
        WxF3 = wpool.tile([128, 32, H], BF16, name="WxF3")
        finTau = wpool.tile([64, B, H], BF16, name="finTau")  # [tau, b, H]
        finT = wpool.tile([128, T, B], BF16, name="finT")     # [H, t, b]
        midAB = wpool.tile([66, 2, T * B], BF16, name="midAB")
        mid2T = wpool.tile([65, B, C2], BF16, name="mid2T")
        zeros128 = wpool.tile([128, 128], F32, name="zeros128")
        ones1 = wpool.tile([1, 64], BF16, name="ones1")
        outsb = wpool.tile([B, 18], F32, name="outsb")
        zbf = wpool.tile([128, 64], BF16, name="zbf")

        nc.vector.memset(zeros128[:], 0.0)
        nc.vector.memset(ones1[:], 1.0)
        nc.vector.memset(zbf[:], 0.0)
        nc.vector.memset(mid2T[64:65, :, :], 1.0)
        nc.vector.memset(X2[0:64, 64, :], PAD_NEG)
        nc.vector.memset(X1[64:128, 8, :], PAD_NEG)
        # midAB rows 64/65: slot0 = [pad-carrier, ones-bias-row] (pad row
        # multiplies the ~0 pad attention weight; weights row 64 are zero);
        # slot1 row 64 = label channel, row 65 = zeros.
        nc.vector.memset(midAB[64:66, 0, :], 1.0)
        nc.vector.memset(midAB[64:66, 1, :], 0.0)
        nc.gpsimd.dma_start(mid2T[0:64, :, 128:129], dr["labelT"][:])
        nc.gpsimd.dma_start(midAB[64:65, 1, :], dr["labelT"][:])

        if only_stages != (1, 2, 3):
            # profiling variants: init tiles a skipped stage would have written
            nc.vector.memset(finT[:], 0.1)
            nc.vector.memset(finTau[:], 0.1)
            nc.vector.memset(midAB[:], 0.1)
            nc.vector.memset(mid2T[:], 0.1)
            nc.vector.memset(X2[:], 0.1)
            nc.vector.memset(X1[:], 0.1)
            nc.vector.memset(WxF3[:], 0.1)
            nc.vector.memset(outsb[:], 0.0)

        # ---------- X1 build ----------
        with tc.tile_pool(name="xb1", space="PSUM", bufs=1) as xb:
            x1ps = xb.tile([128, 9, T], F32, name="x1ps")
            for c in range(NF):
                g, ch = (0, c) if c < 9 else (1, c - 9)
                rows = slice(g * 64, g * 64 + 64)
                nc.tensor.matmul(x1ps[rows, ch, :],
                                 sb["inpT"][:, c * B:(c + 1) * B],
                                 sb["Wi1R"][:], start=True, stop=True)
            nc.vector.tensor_copy(X1[0:64, :, :], x1ps[0:64, :, :])
            nc.scalar.copy(X1[64:128, 0:8, :], x1ps[64:128, 0:8, :])

        # ================= helpers =================
        def lstm_hmajor(gps, chains, c_old, pool):
            # One complete accumulation group per gate: interleaved open
            # groups on one psum tile lose the earlier groups' partials
            # (each later start wipes them), so emit start..stop per gate.
            for k in range(4):
                for j, (Wm, xin) in enumerate(chains):
                    nc.tensor.matmul(gps[:, k, :], Wm[:, 128 * k:128 * (k + 1)],
                                     xin, start=(j == 0),
                                     stop=(j == len(chains) - 1))
            ta = pool.tile([128, 4, 64], F32, name="ta", tag="ta", bufs=2)
            nc.scalar.activation(ta[:], gps[:], AF.Tanh)
            u = pool.tile([128, 64], F32, name="u", tag="u", bufs=2)
            v2 = pool.tile([128, 64], F32, name="v2", tag="v2", bufs=2)
            # u = (tanh(i/2)+1)*tanh(g) = 2*sig(i)*tanh(g)
            nc.vector.scalar_tensor_tensor(u[:], ta[:, 0, :], 1.0,
                                           ta[:, 3, :], op0=OP.add, op1=OP.mult)
            # v = (tanh(f/2)+1)*cS = 4*sig(f)*c
            nc.vector.scalar_tensor_tensor(v2[:], ta[:, 1, :], 1.0,
                                           c_old[:], op0=OP.add, op1=OP.mult)
            # cS_new = v/2 + u = 2*c_new
            c_new = pool.tile([128, 64], F32, name="cN", tag="cN", bufs=2)
            nc.vector.scalar_tensor_tensor(c_new[:], v2[:], 0.5,
                                           u[:], op0=OP.mult, op1=OP.add)
            tcel = pool.tile([128, 64], F32, name="tcel", tag="tcel", bufs=2)
            nc.scalar.activation(tcel[:], c_new[:], AF.Tanh, scale=0.5)
            # hS_new = (tanh(o/2)+1)*tanh(c) = 2*h_new
            h_new = pool.tile([128, 64], BF16, name="hN", tag="hN", bufs=2)
            nc.vector.scalar_tensor_tensor(h_new[:], ta[:, 2, :], 1.0,
                                           tcel[:], op0=OP.add, op1=OP.mult)
            c_bf = pool.tile([128, 64], BF16, name="cbf", tag="cbf", bufs=2)
            nc.vector.tensor_copy(c_bf[:], c_new[:])
            return h_new, c_new, c_bf

        def softmax_nomax(score, pool, ppool, nch, ptag="tps", ones_col=False):
            # score pad slots (if any) must already be ~-30 so exp ~ 0;
            # accum_out fuses the per-partition sum into the exp pass.
            # ones_col appends an all-ones column that the aT transposes turn
            # into a ones row (carrier for the folded gate bias).
            ex = pool.tile([128, nch], F32, name="ex", tag="sm_ex", bufs=2)
            zs = pool.tile([128, 1], F32, name="zs", tag="sm_zs", bufs=2)
            nc.scalar.activation(ex[:], score[:], AF.Exp, accum_out=zs[:])
            zps = ppool.tile([128, 1], F32, name="zps", tag=ptag,
                             bufs=4 if ptag == "tps" else 3)
            nc.tensor.matmul(zps[:], sb["foldDup"][:], zs[:], start=True, stop=True)
            zr = pool.tile([128, 1], F32, name="zr", tag="sm_zr", bufs=2)
            nc.vector.reciprocal(zr[:], zps[:])
            ext = 1 if ones_col else 0
            a = pool.tile([128, nch + ext], BF16, name="a", tag="sm_a", bufs=2)
            if ones_col:
                nc.gpsimd.memset(a[:, nch:nch + 1], 1.0)
            nc.vector.tensor_scalar_mul(a[:, 0:nch], ex[:], zr[:])
            return a

        def tree_to(dst, src, pool, tag, nch, ntau, eng=None):
            """sum src [128, nch, ntau] over tau into dst [128, nch] slice."""
            eng = eng or nc.vector
            nb = 1
            cur, n, lvl = src, ntau, 0
            while n > 2:
                n //= 2
                nxt = pool.tile([128, nch, n], BF16, name=f"{tag}_{lvl}",
                                tag=f"{tag}_{lvl}", bufs=nb)
                eng.tensor_add(nxt[:], cur[:, :, 0:n], cur[:, :, n:2 * n])
                cur, lvl = nxt, lvl + 1
            eng.tensor_add(dst.unsqueeze(-1), cur[:, :, 0:1], cur[:, :, 1:2])

        def score_chunked(Xs, esb, vdup, nch, ntau, sp, tag, pad_neg=None,
                          nchunks=2, pool_ch=0):
            """returns score [128, nch] bf16; chunks over ch for engine overlap.
            The last pool_ch channels run on GPSIMD to offload the DVE."""
            score = sp.tile([128, nch], BF16, name="score", tag=f"{tag}_score",
                            bufs=2)
            nd = nch - pool_ch
            if nchunks == 1:
                bounds = ((0, nd),)
            elif isinstance(nchunks, float):
                cut = max(1, min(nd - 1, int(round(nd * nchunks))))
                bounds = ((0, cut), (cut, nd))
            elif nchunks == 2:
                half = (nd + 1) // 2
                bounds = ((0, half), (half, nd))
            else:
                q = max(1, nd // nchunks)
                cuts = list(range(0, nd, q))
                bounds = tuple((lo, min(lo + q, nd)) for lo in cuts)
            if pool_ch:
                bounds = bounds + ((nd, nch),)
            for lo, hi in bounds:
                w = hi - lo
                eng = nc.gpsimd if (pool_ch and lo == nd) else nc.vector
                nb = 1
                scA = sp.tile([128, w, ntau], BF16, name="scA",
                              tag=f"{tag}_scA{lo}", bufs=nb)
                eng.tensor_add(scA[:], Xs[:, lo:hi, :],
                               esb[:].unsqueeze(1).broadcast_to([128, w, ntau]))
                scT = sp.tile([128, w, ntau], BF16, name="scT",
                              tag=f"{tag}_scT{lo}", bufs=nb)
                nc.scalar.activation(scT[:], scA[:], AF.Tanh)
                scM = sp.tile([128, w, ntau], BF16, name="scM",
                              tag=f"{tag}_scM{lo}", bufs=nb)
                eng.tensor_mul(scM[:], scT[:],
                               vdup[:].unsqueeze(1).broadcast_to([128, w, ntau]))
                tree_to(score[:, lo:hi], scM, sp, f"{tag}_tr{lo}", w, ntau)
            if pad_neg:
                # kill the pad slot before exp (stage1: g1 row-half;
                # stage2: g0 row-half, since g1 slot 64 is the label)
                nc.vector.memset(score[pad_neg[0]:pad_neg[1], nch - 1:nch],
                                 -30.0)
            return score

        # ================= encoder step (H-major LSTM) =================
        def enc_step(t, stage, sp, pp, st):
            if stage == 1:
                Xs, vdup, WeRa, WeRb = X1, sb["vdup1"], sb["We1Ra"], sb["We1Rb"]
                nch = 9
                GH, GXA, GXB = sb["G1H"], sb["G1XA"], sb["G1XB"]
            else:
                Xs, vdup, WeRa, WeRb = X2, sb["vdup2"], sb["We2Ra"], sb["We2Rb"]
                nch = 65
                GH, GXA, GXB = sb["G2H"], sb["G2XA"], sb["G2XB"]
            h_old, c_old, cbf_old = st["h"], st["c"], st["cbf"]

            eps = pp.tile([128, T], F32, name="eps", tag="eps", bufs=2)
            for gb in (0, 64):
                o = eps[gb:gb + 64, :]
                nc.tensor.matmul(o, h_old[:], WeRa[:], start=True, stop=False)
                nc.tensor.matmul(o, cbf_old[:], WeRb[:], start=False, stop=True)
            esb = sp.tile([128, T], BF16, name="esb", tag="esb", bufs=2)
            nc.vector.tensor_copy(esb[:], eps[:])
            if st.get("store") is not None:
                st["store"]()
                st["store"] = None

            gps = pp.tile([128, 4, 64], F32, name="gps", tag="gps", bufs=2)

            if skip_score:
                score = sp.tile([128, nch], BF16, name="score", tag="e_score", bufs=2)
                nc.vector.memset(score[:], 0.1)
            else:
                score = score_chunked(Xs, esb, vdup, nch, T, sp, "e",
                                      pad_neg=(64, 128) if stage == 1 else (0, 64),
                                      nchunks=split,
                                      pool_ch=0 if stage == 1 else pool2)
            a = softmax_nomax(score, sp, pp, nch, ones_col=True)

            # both attention-weight transposes land in one psum tile (free
            # slots 0/1); a's extra ones column becomes the bias-carrier row
            # and the pad column transposes to a ~0 row, so the fused x-mul
            # reads no psum garbage.
            aTA = pp.tile([nch + 1, 64], BF16, name="aTA", tag="tps", bufs=4)
            nc.tensor.transpose(aTA[:], a[0:64, :], sb["I128bf"][0:64, 0:64])
            aTB = pp.tile([nch + 1, 64], BF16, name="aTB", tag="tps", bufs=4)
            nc.tensor.transpose(aTB[:], a[64:128, :], sb["I128bf"][64:128, 64:128])

            # attention-input muls; the ones column of `a` arrives here as a
            # ones row carrying the folded gate bias through GXA
            xAB = sp.tile([nch + 1, 2, 64], BF16, name="xAB", tag="xA", bufs=2)
            if stage == 1:
                nc.vector.tensor_mul(xAB[:, 0, :],
                                     sb["inpCT2"][:, 0, t * B:(t + 1) * B],
                                     aTA[:])
                nc.vector.tensor_mul(xAB[:, 1, :],
                                     sb["inpCT2"][:, 1, t * B:(t + 1) * B],
                                     aTB[:])
                nkb = nch - 1
            else:
                nc.vector.tensor_mul(xAB[:, 0, :],
                                     midAB[:, 0, t * B:(t + 1) * B], aTA[:])
                nc.vector.tensor_mul(xAB[:, 1, :],
                                     midAB[:, 1, t * B:(t + 1) * B], aTB[:])
                nkb = nch
            h_new, c_new, c_bf = lstm_hmajor(
                gps, [(GXA, xAB[:, 0, :]), (GH, h_old[:]),
                      (GXB, xAB[0:nkb, 1, :])], c_old, sp)

            # h^T + time-major stores: deferred past the next step's esb so
            # the ACT/DVE queues stay clear for the critical path.
            def store(t=t, stage=stage, h_new=h_new):
                tp = pp.tile([64, 128], BF16, name="tp", tag="tps", bufs=4)
                nc.tensor.transpose(tp[:], h_new[:], sb["I128bf"][:])
                hTb = sp.tile([64, 128], BF16, name="hTb", tag="hTb", bufs=2)
                nc.scalar.copy(hTb[:], tp[:])
                if stage == 1:
                    nc.vector.tensor_copy(midAB[0:64, 0, t * B:(t + 1) * B],
                                          h_new[0:64, :])
                    nc.sync.dma_start(midAB[0:64, 1, t * B:(t + 1) * B],
                                      h_new[64:128, :])
                    nc.sync.dma_start(mid2T[t:t + 1, :, 0:128], hTb[:])
                else:
                    nc.vector.tensor_copy(finT[:, t, :], h_new[:])
                    nc.sync.dma_start(finTau[t:t + 1, :, :], hTb[:])
            st["store"] = store
            st["h"], st["c"], st["cbf"] = h_new, c_new, c_bf

        # ---------- stage 1 ----------
        with tc.tile_pool(name="s1sp", bufs=2) as sp, \
             tc.tile_pool(name="s1pp", space="PSUM", bufs=2) as pp:
            st = {"h": zbf[:], "c": zeros128[:, 0:64], "cbf": zbf[:]}
            for t in range(T if 1 in only_stages else 0):
                enc_step(t, 1, sp, pp, st)
            if st.get("store") is not None:
                st["store"]()

        # ---------- X2 build ----------
        with tc.tile_pool(name="xb2", space="PSUM", bufs=2) as xb2:
            for r in range(4):
                x2ps = xb2.tile([128, 16, T], F32, name="x2ps", tag="x2ps", bufs=2)
                for k in range(16):
                    ch = r * 16 + k
                    nc.tensor.matmul(x2ps[0:64, k, :], mid2T[:, :, ch],
                                     sb["Wi2R"][:], start=True, stop=True)
                    nc.tensor.matmul(x2ps[64:128, k, :], mid2T[:, :, 64 + ch],
                                     sb["Wi2R"][:], start=True, stop=True)
                nc.vector.tensor_copy(X2[:, r * 16:(r + 1) * 16, :], x2ps[:])
            x2ps2 = xb2.tile([64, T], F32, name="x2ps2", tag="x2ps2", bufs=1)
            nc.tensor.matmul(x2ps2[:], mid2T[:, :, 128], sb["Wi2R"][:],
                             start=True, stop=True)
            nc.vector.tensor_copy(X2[64:128, 64, :], x2ps2[:])

        # ---------- stage 2 ----------
        with tc.tile_pool(name="s2sp", bufs=2) as sp, \
             tc.tile_pool(name="s2pp", space="PSUM", bufs=2) as pp:
            st = {"h": zbf[:], "c": zeros128[:, 0:64], "cbf": zbf[:]}
            for t in range(T if 2 in only_stages else 0):
                enc_step(t, 2, sp, pp, st)
            if st.get("store") is not None:
                st["store"]()

        # ---------- WxF build ----------
        with tc.tile_pool(name="wxb", space="PSUM", bufs=2) as wb:
            for r in range(16):
                g0, sl0 = divmod(r * 4, 32)
                rows = slice(g0 * 64, g0 * 64 + 64)
                wxps = wb.tile([128, 4, H], F32, name="wxps", tag="wxps", bufs=2)
                for j in range(4):
                    nc.tensor.matmul(wxps[rows, j, :], finT[:, r * 4 + j, :],
                                     sb["WxR"][:], start=True, stop=True)
                if r % 2 == 0:
                    nc.vector.tensor_copy(WxF3[rows, sl0:sl0 + 4, :], wxps[rows, :, :])
                else:
                    nc.scalar.copy(WxF3[rows, sl0:sl0 + 4, :], wxps[rows, :, :])

        # ---------- stage 3 (H-major LSTM + PE context einsum) ----------
        with tc.tile_pool(name="s3sp", bufs=2) as sp, \
             tc.tile_pool(name="s3pp", space="PSUM", bufs=2) as pp:
            outps = pp.tile([64, 18], F32, name="outps", bufs=1) if 3 in only_stages else None
            h_old, c_old, cbf_old = zbf[:], zeros128[:, 0:64], zbf[:]
            for t in range(TD if 3 in only_stages else 0):
                eps = pp.tile([128, H], F32, name="e3ps", tag="eps3", bufs=2)
                for gb in (0, 64):
                    o = eps[gb:gb + 64, :]
                    nc.tensor.matmul(o, ones1[:], sb["Wxb"][:], start=True, stop=False)
                    nc.tensor.matmul(o, h_old[:], sb["WhRa"][:],
                                     start=False, stop=False)
                    nc.tensor.matmul(o, cbf_old[:], sb["WhRb"][:],
                                     start=False, stop=True)
                esb = sp.tile([128, H], BF16, name="e3sb", tag="esb3", bufs=2)
                nc.vector.tensor_copy(esb[:], eps[:])

                gps = pp.tile([128, 4, 64], F32, name="g3ps", tag="g3ps", bufs=2)

                if skip_score:
                    score = sp.tile([128, 32], BF16, name="score", tag="d_score", bufs=2)
                    nc.vector.memset(score[:], 0.1)
                else:
                    score = score_chunked(WxF3, esb, sb["vdup3"], 32, H, sp, "d",
                                          nchunks=split3 if split3 is not None else split,
                                          pool_ch=pool3)
                a = softmax_nomax(score, sp, pp, 32, ptag="tps3")

                # a^T [tau, b] then din[H, b] via 64 per-b matmuls
                aTp = pp.tile([32, 64], BF16, name="aTp", tag="tps3", bufs=3)
                nc.tensor.transpose(aTp[:], a[0:64, :], sb["I128bf"][0:64, 0:64])
                aTp2 = pp.tile([32, 64], BF16, name="aTp2", tag="tps3", bufs=3)
                nc.tensor.transpose(aTp2[:], a[64:128, :], sb["I128bf"][64:128, 64:128])
                aTbf = sp.tile([64, 64], BF16, name="aTbf", tag="aTbf", bufs=2)
                nc.vector.tensor_copy(aTbf[0:32, :], aTp[:])
                nc.vector.tensor_copy(aTbf[32:64, :], aTp2[:])
                dinps = pp.tile([128, 64], F32, name="dinps", tag="tps3", bufs=3)
                for b in range(64):
                    nc.tensor.matmul(dinps[:, b:b + 1], finTau[:, b, :],
                                     aTbf[:, b:b + 1], start=True, stop=True)
                dinbf = sp.tile([128, 64], BF16, name="dinbf", tag="dinbf", bufs=2)
                nc.vector.tensor_copy(dinbf[:], dinps[:])

                h_new, c_new, c_bf = lstm_hmajor(
                    gps, [(sb["bdrow"], ones1[:]), (sb["GdH"], h_old[:]),
                          (sb["GdX"], dinbf[:])], c_old, sp)
                h_old, c_old, cbf_old = h_new, c_new, c_bf

                if t >= TD - 18:
                    j = t - (TD - 18)
                    nc.tensor.matmul(outps[:, j:j + 1], h_new[:], sb["regw"][:],
                                     start=True, stop=True)

            if 3 in only_stages:
                nc.vector.tensor_copy(outsb[:], outps[:])
            nc.sync.dma_start(out_d[:], outsb[:])

        wpool.release()

    nc.compile()
    return nc


_NC_CACHE = {}


def kernel(**inputs):
    if "nc" not in _NC_CACHE:
        _NC_CACHE["nc"] = build_nc()
    nc = _NC_CACHE["nc"]
    w = prep_weights({k: np.asarray(v) for k, v in inputs.items()})
    in_maps = []
    for core in range(N_CORES):
        m = dict(w)
        m.update(prep_core_inputs(inputs, core))
        in_maps.append(m)
    res = run_bass_kernel_spmd(nc, in_maps, list(range(N_CORES)))
    out = np.concatenate([res.results[c]["out"] for c in range(N_CORES)], axis=0)
    out = out + np.asarray(inputs["reg_b"])[0]
    return out.astype(np.float32)


# revision 26
# speedup vs baseline: 3.9307x; 3.9307x over previous
# DSTP-RNN Trainium2 kernel: 8-core pure data parallel (batch 512 -> 64/core).
#
# Restructuring summary (validated numerically vs fp32 ref):
#  - "Score" tensors are b-major: partitions = (g, b) with g in {0,1} a
#    channel-group split, b = 64 local batch rows; free dims = (ch, tau).
#  - Per-step attention score: DVE broadcast-add of e, ACT tanh, DVE mul by
#    replicated v, DVE pairwise tree-reduce over tau (all bf16).
#  - Softmax without max-subtraction (scores are small); channel-group fold
#    and per-partition normalizer duplication via tiny PE matmuls.
#  - LSTM is H-major: gates in psum [128H, 4, 64b] built from per-gate
#    matmul chains (weights are the moving operand, all bf16 => 1 cyc/row);
#    h/c states stay [128H, 64b] so no per-step transposes are needed.
#    Gate order host-permuted to [i,f,o | g]; sigmoid via tanh(x/2) with the
#    0.5 folded into weights; states doubled (hS=2h, cS=2c).
#  - Stage-3 context vector din = sum_tau a*final via 64 per-b tiny PE
#    matmuls (stationary = final_b, moving = a_b column), out free size 1.
#  - All cross-partition movement via PE (transpose matmuls with identity,
#    fold/dup matmuls with 0/1 matrices); DVE/ACT stay lane-aligned.
import numpy as np
import ml_dtypes

import concourse.bacc as bacc
import concourse.mybir as mybir
import concourse.tile as tile
from concourse.bass_utils import run_bass_kernel_spmd

F32 = mybir.dt.float32
BF16 = mybir.dt.bfloat16
AX = mybir.AxisListType
OP = mybir.AluOpType
AF = mybir.ActivationFunctionType

N_CORES = 8
B = 64      # batch per core
T = 64      # encoder length
H = 128
TD = 24     # decoder steps (T_DEC + 6)
NF = 17     # driving series count
C2 = 129    # stage-2 channels (H + label)
COLS = np.array(list(range(14)) + list(range(15, 18)))
PAD_NEG = -20.0   # pad channel fill (tanh -> -1; excluded from softmax sums)


def _perm_cols(w):
    # torch gate order (i,f,g,o) -> (i,f,o,g): sigmoid block contiguous
    i, f, g, o = np.split(w, 4, axis=-1)
    return np.concatenate([i, f, o, g], axis=-1)


def _bf(x):
    return np.ascontiguousarray(np.asarray(x).astype(ml_dtypes.bfloat16))


def _f32(x):
    return np.ascontiguousarray(np.asarray(x).astype(np.float32))


def prep_weights(inp):
    w = {}
    w["Wi1R"] = _bf(np.concatenate([inp["Wi_w"].T, inp["Wi_b"][None, :]], 0))
    w["Wi2R"] = _bf(np.concatenate([inp["Wi2_w"].T * 0.5, inp["Wi2_b"][None, :]], 0))
    w["We1R"] = _bf(inp["We_w"].T * 0.5)
    w["We2R"] = _bf(inp["We2_w"].T * 0.5)
    w["WhR"] = _bf(inp["Wh_w"].T * 0.5)
    w["WxR"] = _bf(inp["Wx_w"].T * 0.5)
    w["Wxb"] = _bf(inp["Wx_b"][None, :])

    # ISO: sigmoid gates computed as tanh(x/2) -> pre-scale i,f,o cols by 0.5.
    # States are stored doubled (hS=2h, cS=2c), so weight blocks consuming
    # h/c/mid/din get an extra 0.5.
    ISO = np.concatenate([0.5 * np.ones(384), np.ones(128)]).astype(np.float32)
    g1x = _perm_cols(inp["Wih1"].T) * ISO
    b1 = _perm_cols((inp["bih1"] + inp["bhh1"])[None, :]) * ISO
    w["G1XA"] = _bf(np.concatenate([g1x[0:9], b1], 0))
    w["G1XB"] = _bf(g1x[9:17])
    w["G1H"] = _bf(_perm_cols(inp["Whh1"].T) * ISO * 0.5)

    # stage-2 channel order: chA = [h0..h63, label], chB = [h64..h127]
    g2x = _perm_cols(inp["Wih2"].T) * ISO * 0.5
    b2 = _perm_cols((inp["bih2"] + inp["bhh2"])[None, :]) * ISO
    w["G2XA"] = _bf(np.concatenate([g2x[0:64], np.zeros((1, 512), np.float32), b2], 0))
    w["G2XB"] = _bf(np.concatenate([g2x[64:128], g2x[128:129]], 0))
    w["G2H"] = _bf(_perm_cols(inp["Whh2"].T) * ISO * 0.5)

    w["GdX"] = _bf(_perm_cols(inp["Wihd"].T) * ISO * 0.5)
    w["GdH"] = _bf(_perm_cols(inp["Whhd"].T) * ISO * 0.5)
    w["bdrow"] = _bf(_perm_cols((inp["bihd"] + inp["bhhd"])[None, :]) * ISO)

    w["vdup1"] = _bf(np.broadcast_to(inp["Vd_w"][0][None, :], (128, T)))
    w["vdup2"] = _bf(np.broadcast_to(inp["Vd2_w"][0][None, :], (128, T)))
    w["vdup3"] = _bf(np.broadcast_to(inp["V_w"][0][None, :], (128, H)))
    w["regw"] = _bf(inp["reg_w"][0][:, None] * 0.5)

    eye = np.eye(64, dtype=np.float32)
    w["I64dup"] = _f32(np.concatenate([eye, eye], 0))
    w["I128bf"] = _bf(np.eye(128, dtype=np.float32))
    foldDup = (np.arange(128)[:, None] % 64 == np.arange(128)[None, :] % 64)
    w["foldDup"] = _f32(foldDup.astype(np.float32))
    return w


def prep_core_inputs(inp, core):
    b0, b1 = core * B, (core + 1) * B
    x = np.asarray(inp["input_p_q"])[b0:b1, :T, :][:, :, COLS]   # [64,64,17]
    lab = np.asarray(inp["label_p"])[b0:b1, :T]                  # [64,64]
    d = {}
    inpT = np.ones((65, NF * B), np.float32)
    inpT[:64] = x.transpose(1, 2, 0).reshape(64, NF * B)         # [t, (c,b)]
    d["inpT"] = _bf(inpT)
    ct = x.transpose(2, 1, 0).reshape(NF, T * B)                 # [c, (t,b)]
    ct2 = np.zeros((10, 2, T * B), np.float32)
    ct2[0:9, 0] = ct[0:9]
    ct2[0:8, 1] = ct[9:17]
    ct2[9, 0] = 1.0
    d["inpCT2"] = _bf(ct2.reshape(10, 2 * T * B))
    d["labelT"] = _f32(lab.T * 2.0)                                    # [t, b]
    return d


DRAM_SPECS = {
    "inpT": ([65, NF * B], BF16), "inpCT2": ([10, 2 * T * B], BF16),
    "labelT": ([T, B], F32),
    "Wi1R": ([65, 64], BF16), "Wi2R": ([65, 64], BF16),
    "We1R": ([256, 64], BF16), "We2R": ([256, 64], BF16),
    "WhR": ([256, 128], BF16), "WxR": ([128, 128], BF16), "Wxb": ([1, 128], BF16),
    "G1XA": ([10, 512], BF16), "G1XB": ([8, 512], BF16), "G1H": ([128, 512], BF16),
    "G2XA": ([66, 512], BF16), "G2XB": ([65, 512], BF16), "G2H": ([128, 512], BF16),
    "GdX": ([128, 512], BF16), "GdH": ([128, 512], BF16), "bdrow": ([1, 512], BF16),
    "vdup1": ([128, T], BF16), "vdup2": ([128, T], BF16), "vdup3": ([128, H], BF16),
    "regw": ([128, 1], BF16), "I64dup": ([128, 64], F32), "I128bf": ([128, 128], BF16),
    "foldDup": ([128, 128], F32),
}


def build_nc(num_devices=N_CORES, skip_score=False, skip_tail=False, only_stages=(1, 2, 3), split=0.42, split1=None, split3=None, pool2=0, pool3=0):
    nc = bacc.Bacc("TRN2", target_bir_lowering=False, debug=False,
                   num_devices=num_devices)
    dr = {}
    for name, (shape, dt) in DRAM_SPECS.items():
        dr[name] = nc.dram_tensor(name, shape, dt, kind="ExternalInput").ap()
    out_d = nc.dram_tensor("out", [B, 18], F32, kind="ExternalOutput").ap()

    with tile.TileContext(nc) as tc:
        # ---------- persistent SBUF ----------
        wpool = tc.alloc_tile_pool(name="wpool", bufs=1)
        sb = {}
        for name, (shape, dt) in DRAM_SPECS.items():
            if shape[0] > 128:
                assert shape[0] == 256
                for half, suf in ((0, "a"), (1, "b")):
                    key = name + suf
                    sb[key] = wpool.tile([128, shape[1]], dt, name=f"sb_{key}")
                    nc.sync.dma_start(sb[key][:], dr[name][128 * half:128 * (half + 1), :])
            elif name == "inpCT2":
                sb[name] = wpool.tile([10, 2, T * B], dt, name=f"sb_{name}")
                nc.sync.dma_start(sb[name][:, 0, :], dr[name][:, 0:T * B])
                nc.sync.dma_start(sb[name][:, 1, :], dr[name][:, T * B:])
            else:
                sb[name] = wpool.tile(shape, dt, name=f"sb_{name}")
                nc.sync.dma_start(sb[name][:], dr[name][:])

        X1 = wpool.tile([128, 9, T], BF16, name="X1")
        X2 = wpool.tile([128, 65, T], BF16, name="X2")
        WxF3 = wpool.tile([128, 32, H], BF16, name="WxF3")
        finTau = wpool.tile([64, B, H], BF16, name="finTau")  # [tau, b, H]
        finT = wpool.tile([128, T, B], BF16, name="finT")     # [H, t, b]
        midAB = wpool.tile([66, 2, T * B], BF16, name="midAB")
        mid2T = wpool.tile([65, B, C2], BF16, name="mid2T")
        zeros128 = wpool.tile([128, 128], F32, name="zeros128")
        ones1 = wpool.tile([1, 64], BF16, name="ones1")
        outsb = wpool.tile([B, 18], F32, name="outsb")
        zbf = wpool.tile([128, 64], BF16, name="zbf")

        nc.vector.memset(zeros128[:], 0.0)
        nc.vector.memset(ones1[:], 1.0)
        nc.vector.memset(zbf[:], 0.0)
        nc.vector.memset(mid2T[64:65, :, :], 1.0)
        nc.vector.memset(X2[0:64, 64, :], PAD_NEG)
        nc.vector.memset(X1[64:128, 8, :], PAD_NEG)
        # midAB rows 64/65: slot0 = [pad-carrier, ones-bias-row] (pad row
        # multiplies the ~0 pad attention weight; weights row 64 are zero);
        # slot1 row 64 = label channel, row 65 = zeros.
        nc.vector.memset(midAB[64:66, 0, :], 1.0)
        nc.vector.memset(midAB[64:66, 1, :], 0.0)
        nc.gpsimd.dma_start(mid2T[0:64, :, 128:129], dr["labelT"][:])
        nc.gpsimd.dma_start(midAB[64:65, 1, :], dr["labelT"][:])

        if only_stages != (1, 2, 3):
            # profiling variants: init tiles a skipped stage would have written
            nc.vector.memset(finT[:], 0.1)
            nc.vector.memset(finTau[:], 0.1)
            nc.vector.memset(midAB[:], 0.1)
            nc.vector.memset(mid2T[:], 0.1)
            nc.vector.memset(X2[:], 0.1)
            nc.vector.memset(X1[:], 0.1)
            nc.vector.memset(WxF3[:], 0.1)
            nc.vector.memset(outsb[:], 0.0)

        # ---------- X1 build ----------
        with tc.tile_pool(name="xb1", space="PSUM", bufs=1) as xb:
            x1ps = xb.tile([128, 9, T], F32, name="x1ps")
            for c in range(NF):
                g, ch = (0, c) if c < 9 else (1, c - 9)
                rows = slice(g * 64, g * 64 + 64)
                nc.tensor.matmul(x1ps[rows, ch, :],
                                 sb["inpT"][:, c * B:(c + 1) * B],
                                 sb["Wi1R"][:], start=True, stop=True)
            nc.vector.tensor_copy(X1[0:64, :, :], x1ps[0:64, :, :])
            nc.scalar.copy(X1[64:128, 0:8, :], x1ps[64:128, 0:8, :])

        # ================= helpers =================
        def lstm_hmajor(gps, chains, c_old, pool):
            # One complete accumulation group per gate: interleaved open
            # groups on one psum tile lose the earlier groups' partials
            # (each later start wipes them), so emit start..stop per gate.
            for k in range(4):
                for j, (Wm, xin) in enumerate(chains):
                    nc.tensor.matmul(gps[:, k, :], Wm[:, 128 * k:128 * (k + 1)],
                                     xin, start=(j == 0),
                                     stop=(j == len(chains) - 1))
            ta = pool.tile([128, 4, 64], F32, name="ta", tag="ta", bufs=2)
            nc.scalar.activation(ta[:], gps[:], AF.Tanh)
            u = pool.tile([128, 64], F32, name="u", tag="u", bufs=2)
            v2 = pool.tile([128, 64], F32, name="v2", tag="v2", bufs=2)
            # u = (tanh(i/2)+1)*tanh(g) = 2*sig(i)*tanh(g)
            nc.vector.scalar_tensor_tensor(u[:], ta[:, 0, :], 1.0,
                                           ta[:, 3, :], op0=OP.add, op1=OP.mult)
            # v = (tanh(f/2)+1)*cS = 4*sig(f)*c
            nc.vector.scalar_tensor_tensor(v2[:], ta[:, 1, :], 1.0,
                                           c_old[:], op0=OP.add, op1=OP.mult)
            # cS_new = v/2 + u = 2*c_new
            c_new = pool.tile([128, 64], F32, name="cN", tag="cN", bufs=2)
            nc.vector.scalar_tensor_tensor(c_new[:], v2[:], 0.5,
                                           u[:], op0=OP.mult, op1=OP.add)
            tcel = pool.tile([128, 64], F32, name="tcel", tag="tcel", bufs=2)
            nc.scalar.activation(tcel[:], c_new[:], AF.Tanh, scale=0.5)
            # hS_new = (tanh(o/2)+1)*tanh(c) = 2*h_new
            h_new = pool.tile([128, 64], BF16, name="hN", tag="hN", bufs=2)
            nc.vector.scalar_tensor_tensor(h_new[:], ta[:, 2, :], 1.0,
                                           tcel[:], op0=OP.add, op1=OP.mult)
            c_bf = pool.tile([128, 64], BF16, name="cbf", tag="cbf", bufs=2)
            nc.vector.tensor_copy(c_bf[:], c_new[:])
            return h_new, c_new, c_bf

        def softmax_nomax(score, pool, ppool, nch, ptag="tps", ones_col=False):
            # score pad slots (if any) must already be ~-30 so exp ~ 0;
            # accum_out fuses the per-partition sum into the exp pass.
            # ones_col appends an all-ones column that the aT transposes turn
            # into a ones row (carrier for the folded gate bias).
            ex = pool.tile([128, nch], F32, name="ex", tag="sm_ex", bufs=2)
            zs = pool.tile([128, 1], F32, name="zs", tag="sm_zs", bufs=2)
            nc.scalar.activation(ex[:], score[:], AF.Exp, accum_out=zs[:])
            zps = ppool.tile([128, 1], F32, name="zps", tag=ptag,
                             bufs=4 if ptag == "tps" else 3)
            nc.tensor.matmul(zps[:], sb["foldDup"][:], zs[:], start=True, stop=True)
            zr = pool.tile([128, 1], F32, name="zr", tag="sm_zr", bufs=2)
            nc.vector.reciprocal(zr[:], zps[:])
            ext = 1 if ones_col else 0
            a = pool.tile([128, nch + ext], BF16, name="a", tag="sm_a", bufs=2)
            if ones_col:
                nc.gpsimd.memset(a[:, nch:nch + 1], 1.0)
            nc.vector.tensor_scalar_mul(a[:, 0:nch], ex[:], zr[:])
            return a

        def tree_to(dst, src, pool, tag, nch, ntau, eng=None):
            """sum src [128, nch, ntau] over tau into dst [128, nch] slice."""
            eng = eng or nc.vector
            nb = 1
            cur, n, lvl = src, ntau, 0
            while n > 2:
                n //= 2
                nxt = pool.tile([128, nch, n], BF16, name=f"{tag}_{lvl}",
                                tag=f"{tag}_{lvl}", bufs=nb)
                eng.tensor_add(nxt[:], cur[:, :, 0:n], cur[:, :, n:2 * n])
                cur, lvl = nxt, lvl + 1
            eng.tensor_add(dst.unsqueeze(-1), cur[:, :, 0:1], cur[:, :, 1:2])

        def score_chunked(Xs, esb, vdup, nch, ntau, sp, tag, pad_neg=None,
                          nchunks=2, pool_ch=0):
            """returns score [128, nch] bf16; chunks over ch for engine overlap.
            The last pool_ch channels run on GPSIMD to offload the DVE."""
            score = sp.tile([128, nch], BF16, name="score", tag=f"{tag}_score",
                            bufs=2)
            nd = nch - pool_ch
            if nchunks == 1:
                bounds = ((0, nd),)
            elif isinstance(nchunks, tuple):
                cuts, acc = [0], 0.0
                for f in nchunks:
                    acc += f
                    cuts.append(max(1, min(nd, int(round(nd * acc)))))
                cuts.append(nd)
                bounds = tuple((lo, hi) for lo, hi in zip(cuts, cuts[1:])
                               if hi > lo)
            elif isinstance(nchunks, float):
                cut = max(1, min(nd - 1, int(round(nd * nchunks))))
                bounds = ((0, cut), (cut, nd))
            elif nchunks == 2:
                half = (nd + 1) // 2
                bounds = ((0, half), (half, nd))
            else:
                q = max(1, nd // nchunks)
                cuts = list(range(0, nd, q))
                bounds = tuple((lo, min(lo + q, nd)) for lo in cuts)
            if pool_ch:
                bounds = bounds + ((nd, nch),)
            for lo, hi in bounds:
                w = hi - lo
                eng = nc.gpsimd if (pool_ch and lo == nd) else nc.vector
                nb = 1
                scA = sp.tile([128, w, ntau], BF16, name="scA",
                              tag=f"{tag}_scA{lo}", bufs=nb)
                eng.tensor_add(scA[:], Xs[:, lo:hi, :],
                               esb[:].unsqueeze(1).broadcast_to([128, w, ntau]))
                scT = sp.tile([128, w, ntau], BF16, name="scT",
                              tag=f"{tag}_scT{lo}", bufs=nb)
                nc.scalar.activation(scT[:], scA[:], AF.Tanh)
                scM = sp.tile([128, w, ntau], BF16, name="scM",
                              tag=f"{tag}_scM{lo}", bufs=nb)
                eng.tensor_mul(scM[:], scT[:],
                               vdup[:].unsqueeze(1).broadcast_to([128, w, ntau]))
                tree_to(score[:, lo:hi], scM, sp, f"{tag}_tr{lo}", w, ntau)
            if pad_neg:
                # kill the pad slot before exp (stage1: g1 row-half;
                # stage2: g0 row-half, since g1 slot 64 is the label)
                nc.vector.memset(score[pad_neg[0]:pad_neg[1], nch - 1:nch],
                                 -30.0)
            return score

        # ================= encoder step (H-major LSTM) =================
        def enc_step(t, stage, sp, pp, st):
            if stage == 1:
                Xs, vdup, WeRa, WeRb = X1, sb["vdup1"], sb["We1Ra"], sb["We1Rb"]
                nch = 9
                GH, GXA, GXB = sb["G1H"], sb["G1XA"], sb["G1XB"]
            else:
                Xs, vdup, WeRa, WeRb = X2, sb["vdup2"], sb["We2Ra"], sb["We2Rb"]
                nch = 65
                GH, GXA, GXB = sb["G2H"], sb["G2XA"], sb["G2XB"]
            h_old, c_old, cbf_old = st["h"], st["c"], st["cbf"]

            eps = pp.tile([128, T], F32, name="eps", tag="eps", bufs=2)
            for gb in (0, 64):
                o = eps[gb:gb + 64, :]
                nc.tensor.matmul(o, h_old[:], WeRa[:], start=True, stop=False)
                nc.tensor.matmul(o, cbf_old[:], WeRb[:], start=False, stop=True)
            esb = sp.tile([128, T], BF16, name="esb", tag="esb", bufs=2)
            nc.vector.tensor_copy(esb[:], eps[:])
            if st.get("store") is not None:
                st["store"]()
                st["store"] = None

            gps = pp.tile([128, 4, 64], F32, name="gps", tag="gps", bufs=2)

            if skip_score:
                score = sp.tile([128, nch], BF16, name="score", tag="e_score", bufs=2)
                nc.vector.memset(score[:], 0.1)
            else:
                score = score_chunked(Xs, esb, vdup, nch, T, sp, "e",
                                      pad_neg=(64, 128) if stage == 1 else (0, 64),
                                      nchunks=(split1 if split1 is not None else split)
                                      if stage == 1 else split,
                                      pool_ch=0 if stage == 1 else pool2)
            a = softmax_nomax(score, sp, pp, nch, ones_col=True)

            # both attention-weight transposes land in one psum tile (free
            # slots 0/1); a's extra ones column becomes the bias-carrier row
            # and the pad column transposes to a ~0 row, so the fused x-mul
            # reads no psum garbage.
            aTA = pp.tile([nch + 1, 64], BF16, name="aTA", tag="tps", bufs=4)
            nc.tensor.transpose(aTA[:], a[0:64, :], sb["I128bf"][0:64, 0:64])
            aTB = pp.tile([nch + 1, 64], BF16, name="aTB", tag="tps", bufs=4)
            nc.tensor.transpose(aTB[:], a[64:128, :], sb["I128bf"][64:128, 64:128])

            # attention-input muls; the ones column of `a` arrives here as a
            # ones row carrying the folded gate bias through GXA
            xAB = sp.tile([nch + 1, 2, 64], BF16, name="xAB", tag="xA", bufs=2)
            if stage == 1:
                nc.vector.tensor_mul(xAB[:, 0, :],
                                     sb["inpCT2"][:, 0, t * B:(t + 1) * B],
                                     aTA[:])
                nc.vector.tensor_mul(xAB[:, 1, :],
                                     sb["inpCT2"][:, 1, t * B:(t + 1) * B],
                                     aTB[:])
                nkb = nch - 1
            else:
                nc.vector.tensor_mul(xAB[:, 0, :],
                                     midAB[:, 0, t * B:(t + 1) * B], aTA[:])
                nc.vector.tensor_mul(xAB[:, 1, :],
                                     midAB[:, 1, t * B:(t + 1) * B], aTB[:])
                nkb = nch
            h_new, c_new, c_bf = lstm_hmajor(
                gps, [(GXA, xAB[:, 0, :]), (GH, h_old[:]),
                      (GXB, xAB[0:nkb, 1, :])], c_old, sp)

            # h^T + time-major stores: deferred past the next step's esb so
            # the ACT/DVE queues stay clear for the critical path.
            def store(t=t, stage=stage, h_new=h_new):
                tp = pp.tile([64, 128], BF16, name="tp", tag="tps", bufs=4)
                nc.tensor.transpose(tp[:], h_new[:], sb["I128bf"][:])
                hTb = sp.tile([64, 128], BF16, name="hTb", tag="hTb", bufs=2)
                nc.scalar.copy(hTb[:], tp[:])
                if stage == 1:
                    nc.scalar.copy(midAB[0:64, 0, t * B:(t + 1) * B],
                                   h_new[0:64, :])
                    nc.sync.dma_start(midAB[0:64, 1, t * B:(t + 1) * B],
                                      h_new[64:128, :])
                    nc.sync.dma_start(mid2T[t:t + 1, :, 0:128], hTb[:])
                else:
                    nc.scalar.copy(finT[:, t, :], h_new[:])
                    nc.sync.dma_start(finTau[t:t + 1, :, :], hTb[:])
            st["store"] = store
            st["h"], st["c"], st["cbf"] = h_new, c_new, c_bf

        # ---------- stage 1 ----------
        with tc.tile_pool(name="s1sp", bufs=2) as sp, \
             tc.tile_pool(name="s1pp", space="PSUM", bufs=2) as pp:
            st = {"h": zbf[:], "c": zeros128[:, 0:64], "cbf": zbf[:]}
            for t in range(T if 1 in only_stages else 0):
                enc_step(t, 1, sp, pp, st)
            if st.get("store") is not None:
                st["store"]()

        # ---------- X2 build ----------
        with tc.tile_pool(name="xb2", space="PSUM", bufs=2) as xb2:
            for r in range(4):
                x2ps = xb2.tile([128, 16, T], F32, name="x2ps", tag="x2ps", bufs=2)
                for k in range(16):
                    ch = r * 16 + k
                    nc.tensor.matmul(x2ps[0:64, k, :], mid2T[:, :, ch],
                                     sb["Wi2R"][:], start=True, stop=True)
                    nc.tensor.matmul(x2ps[64:128, k, :], mid2T[:, :, 64 + ch],
                                     sb["Wi2R"][:], start=True, stop=True)
                nc.vector.tensor_copy(X2[:, r * 16:(r + 1) * 16, :], x2ps[:])
            x2ps2 = xb2.tile([64, T], F32, name="x2ps2", tag="x2ps2", bufs=1)
            nc.tensor.matmul(x2ps2[:], mid2T[:, :, 128], sb["Wi2R"][:],
                             start=True, stop=True)
            nc.vector.tensor_copy(X2[64:128, 64, :], x2ps2[:])

        # ---------- stage 2 ----------
        with tc.tile_pool(name="s2sp", bufs=2) as sp, \
             tc.tile_pool(name="s2pp", space="PSUM", bufs=2) as pp:
            st = {"h": zbf[:], "c": zeros128[:, 0:64], "cbf": zbf[:]}
            for t in range(T if 2 in only_stages else 0):
                enc_step(t, 2, sp, pp, st)
            if st.get("store") is not None:
                st["store"]()

        # ---------- WxF build ----------
        with tc.tile_pool(name="wxb", space="PSUM", bufs=2) as wb:
            for r in range(16):
                g0, sl0 = divmod(r * 4, 32)
                rows = slice(g0 * 64, g0 * 64 + 64)
                wxps = wb.tile([128, 4, H], F32, name="wxps", tag="wxps", bufs=2)
                for j in range(4):
                    nc.tensor.matmul(wxps[rows, j, :], finT[:, r * 4 + j, :],
                                     sb["WxR"][:], start=True, stop=True)
                if r % 2 == 0:
                    nc.vector.tensor_copy(WxF3[rows, sl0:sl0 + 4, :], wxps[rows, :, :])
                else:
                    nc.scalar.copy(WxF3[rows, sl0:sl0 + 4, :], wxps[rows, :, :])

        # ---------- stage 3 (H-major LSTM + PE context einsum) ----------
        with tc.tile_pool(name="s3sp", bufs=2) as sp, \
             tc.tile_pool(name="s3pp", space="PSUM", bufs=2) as pp:
            outps = pp.tile([64, 18], F32, name="outps", bufs=1) if 3 in only_stages else None
            h_old, c_old, cbf_old = zbf[:], zeros128[:, 0:64], zbf[:]
            for t in range(TD if 3 in only_stages else 0):
                eps = pp.tile([128, H], F32, name="e3ps", tag="eps3", bufs=2)
                for gb in (0, 64):
                    o = eps[gb:gb + 64, :]
                    nc.tensor.matmul(o, ones1[:], sb["Wxb"][:], start=True, stop=False)
                    nc.tensor.matmul(o, h_old[:], sb["WhRa"][:],
                                     start=False, stop=False)
                    nc.tensor.matmul(o, cbf_old[:], sb["WhRb"][:],
                                     start=False, stop=True)
                esb = sp.tile([128, H], BF16, name="e3sb", tag="esb3", bufs=2)
                nc.vector.tensor_copy(esb[:], eps[:])

                gps = pp.tile([128, 4, 64], F32, name="g3ps", tag="g3ps", bufs=2)

                if skip_score:
                    score = sp.tile([128, 32], BF16, name="score", tag="d_score", bufs=2)
                    nc.vector.memset(score[:], 0.1)
                else:
                    score = score_chunked(WxF3, esb, sb["vdup3"], 32, H, sp, "d",
                                          nchunks=split3 if split3 is not None else split,
                                          pool_ch=pool3)
                a = softmax_nomax(score, sp, pp, 32, ptag="tps3")

                # a^T [tau, b] then din[H, b] via 64 per-b matmuls
                aTp = pp.tile([32, 64], BF16, name="aTp", tag="tps3", bufs=3)
                nc.tensor.transpose(aTp[:], a[0:64, :], sb["I128bf"][0:64, 0:64])
                aTp2 = pp.tile([32, 64], BF16, name="aTp2", tag="tps3", bufs=3)
                nc.tensor.transpose(aTp2[:], a[64:128, :], sb["I128bf"][64:128, 64:128])
                aTbf = sp.tile([64, 64], BF16, name="aTbf", tag="aTbf", bufs=2)
                nc.vector.tensor_copy(aTbf[0:32, :], aTp[:])
                nc.vector.tensor_copy(aTbf[32:64, :], aTp2[:])
                dinps = pp.tile([128, 64], F32, name="dinps", tag="tps3", bufs=3)
                for b in range(64):
                    nc.tensor.matmul(dinps[:, b:b + 1], finTau[:, b, :],
                                     aTbf[:, b:b + 1], start=True, stop=True)
                dinbf = sp.tile([128, 64], BF16, name="dinbf", tag="dinbf", bufs=2)
                nc.vector.tensor_copy(dinbf[:], dinps[:])

                h_new, c_new, c_bf = lstm_hmajor(
                    gps, [(sb["bdrow"], ones1[:]), (sb["GdH"], h_old[:]),
                          (sb["GdX"], dinbf[:])], c_old, sp)
                h_old, c_old, cbf_old = h_new, c_new, c_bf

                if t >= TD - 18:
                    j = t - (TD - 18)
                    nc.tensor.matmul(outps[:, j:j + 1], h_new[:], sb["regw"][:],
                                     start=True, stop=True)

            if 3 in only_stages:
                nc.vector.tensor_copy(outsb[:], outps[:])
            nc.sync.dma_start(out_d[:], outsb[:])

        wpool.release()

    nc.compile()
    return nc


_NC_CACHE = {}


def kernel(**inputs):
    if "nc" not in _NC_CACHE:
        _NC_CACHE["nc"] = build_nc()
    nc = _NC_CACHE["nc"]
    w = prep_weights({k: np.asarray(v) for k, v in inputs.items()})
    in_maps = []
    for core in range(N_CORES):
        m = dict(w)
        m.update(prep_core_inputs(inputs, core))
        in_maps.append(m)
    res = run_bass_kernel_spmd(nc, in_maps, list(range(N_CORES)))
    out = np.concatenate([res.results[c]["out"] for c in range(N_CORES)], axis=0)
    out = out + np.asarray(inputs["reg_b"])[0]
    return out.astype(np.float32)
